# revision 2
# baseline (speedup 1.0000x reference)
"""Trainium2 Bass kernel for ragged-sequence growing-prefix softmax attention.

Reference computation (T=131072 tokens, B=1024 ragged segments, D=512):
    s = context @ theta                       # [T]
    e = exp(s - segmax)                       # segment max cancels in the ratio
    out_t = segprefix(e*c)_t / segprefix(e)_t

Device strategy (8 cores, data-parallel over segments):
  - 16 sub-slabs cut at segment boundaries near j*8192 tokens; core c gets
    sub-slabs 2c, 2c+1 as two independent carry chains (hides the serial
    carry latency under DMA).
  - Each chain is processed in 66 tiles of 127 tokens + 1 carry row (row 0).
    A host-built mask M[j,i] = (i>=j) & (i <= end_j) (end_j = last row of
    j's segment in this tile) turns the segmented inclusive prefix sum into
    one 128x128 masked-triangular matmul per tile:  psum = M.T @ [e*x].
  - e*x enters the matmul as a bf16 hi/lo pair (hi = bf16(e*x),
    lo = bf16(e*x - hi)); two accumulating bf16 matmuls reconstruct ~fp32
    precision (~2^-18 rel) at full PE rate.
  - Cross-tile carry: bcol[j] = (end_j == 127) selects rows of the segment
    open at the tile end; bcol.T @ [e*x] is a rank-1 matmul landing the
    running segment sum on partition 0, which is re-injected as row 0 of the
    next tile's rhs (hi/lo split again; exact compensation).
  - den uses the same mask with rhs = e (hi/lo split), out = num * 1/den.
"""
import numpy as np

T = 131072
B = 1024
D = 512
NCORES = 8
NSUB = 16              # sub-slabs (2 per core)
TPT = 127              # tokens per tile (row 0 is the carry row)
SUBTILES = 66          # tiles per sub-slab
NPAD = TPT * SUBTILES  # 8382 padded tokens per sub-slab

_CACHE = {}


def _build_program():
    import concourse.bacc as bacc
    import concourse.tile as tile
    import concourse.mybir as mybir
    from contextlib import ExitStack

    f32 = mybir.dt.float32
    bf16 = mybir.dt.bfloat16
    AF = mybir.ActivationFunctionType
    ALU = mybir.AluOpType

    nc = bacc.Bacc("TRN2", target_bir_lowering=False, debug=False)

    x_d = [nc.dram_tensor(f"x{ch}", [1 + NPAD, D], f32, kind="ExternalInput")
           for ch in range(2)]
    m_d = [nc.dram_tensor(f"masks{ch}", [SUBTILES, 128, 128], bf16,
                          kind="ExternalInput") for ch in range(2)]
    b_d = [nc.dram_tensor(f"bcols{ch}", [128, SUBTILES], bf16,
                          kind="ExternalInput") for ch in range(2)]
    th_d = nc.dram_tensor("thetab", [128, D], f32, kind="ExternalInput")
    y_d = [nc.dram_tensor(f"y{ch}", [NPAD, D], f32, kind="ExternalOutput")
           for ch in range(2)]

    with tile.TileContext(nc) as tc, ExitStack() as ctx:
        cpool = ctx.enter_context(tc.tile_pool(name="consts", bufs=1))
        xpool = ctx.enter_context(tc.tile_pool(name="x", bufs=3))
        mpool = ctx.enter_context(tc.tile_pool(name="mask", bufs=3))
        hpool = ctx.enter_context(tc.tile_pool(name="xehi", bufs=3))
        lpool = ctx.enter_context(tc.tile_pool(name="xelo", bufs=3))
        spool = ctx.enter_context(tc.tile_pool(name="small", bufs=3))
        rpool = ctx.enter_context(tc.tile_pool(name="scr", bufs=2))
        opool = ctx.enter_context(tc.tile_pool(name="out", bufs=3))
        pn_pool = ctx.enter_context(tc.tile_pool(name="pnum", bufs=2, space="PSUM"))
        pd_pool = ctx.enter_context(tc.tile_pool(name="pden", bufs=2, space="PSUM"))
        pc_pool = ctx.enter_context(tc.tile_pool(name="pcar", bufs=2, space="PSUM"))

        thetab = cpool.tile([128, D], f32)
        nc.sync.dma_start(thetab[:], th_d.ap()[:])
        bcol_all = [cpool.tile([128, SUBTILES], bf16, name=f"bcall{ch}",
                               tag=f"bc{ch}") for ch in range(2)]
        for ch in range(2):
            nc.sync.dma_start(bcol_all[ch][:], b_d[ch].ap()[:])

        # per-chain previous-tile psum handles for the carry injection
        prev = [None, None]

        for k in range(SUBTILES):
            for ch in range(2):
                x = xpool.tile([128, D], f32)
                nc.sync.dma_start(x[:], x_d[ch].ap()[TPT * k: TPT * k + 128, :])
                mask = mpool.tile([128, 128], bf16)
                nc.sync.dma_start(mask[:], m_d[ch].ap()[k])

                # s = sum(x * thetab) along free dim
                scr = rpool.tile([128, D], f32)
                nc.gpsimd.tensor_tensor(scr[:], x[:], thetab[:], op=ALU.mult)
                s = spool.tile([128, 1], f32, tag="s")
                nc.vector.tensor_reduce(s[:], scr[:],
                                        axis=mybir.AxisListType.X, op=ALU.add)
                e32 = spool.tile([128, 1], f32, tag="e32")
                nc.scalar.activation(e32[:], s[:], AF.Exp)
                e_hi = spool.tile([128, 1], bf16, tag="ehi")
                nc.scalar.copy(e_hi[:], e32[:])
                e_lo = spool.tile([128, 1], bf16, tag="elo")
                nc.vector.tensor_tensor(e_lo[:], e32[:], e_hi[:], op=ALU.subtract)

                # xe = x * e as a bf16 hi/lo pair
                xe_hi = hpool.tile([128, D], bf16)
                nc.scalar.activation(xe_hi[:], x[:], AF.Copy, scale=e32[:])

                if prev[ch] is not None:
                    pcar_p, pden_p = prev[ch]
                    # inject carry (num) into row 0: hi then compensated lo
                    nc.scalar.copy(xe_hi[0:1, :], pcar_p[:])
                xe_lo = lpool.tile([128, D], bf16)
                nc.vector.scalar_tensor_tensor(
                    xe_lo[:], in0=x[:], scalar=e32[:], in1=xe_hi[:],
                    op0=ALU.mult, op1=ALU.subtract)
                if prev[ch] is not None:
                    pcar_p, pden_p = prev[ch]
                    nc.vector.tensor_tensor(xe_lo[0:1, :], pcar_p[:],
                                            xe_hi[0:1, :], op=ALU.subtract)
                    # carry (den) into e row 0
                    nc.scalar.copy(e_hi[0:1, :], pden_p[0:1, 1:2])
                    nc.vector.tensor_tensor(e_lo[0:1, :], pden_p[0:1, 1:2],
                                            e_hi[0:1, :], op=ALU.subtract)

                bcol = bcol_all[ch][:, k:k + 1]

                pnum = pn_pool.tile([128, D], f32)
                nc.tensor.matmul(pnum[:], lhsT=mask[:], rhs=xe_hi[:],
                                 start=True, stop=False)
                nc.tensor.matmul(pnum[:], lhsT=mask[:], rhs=xe_lo[:],
                                 start=False, stop=True)
                pden = pd_pool.tile([128, 2], f32)
                nc.tensor.matmul(pden[:, 0:1], lhsT=mask[:], rhs=e_hi[:],
                                 start=True, stop=False)
                nc.tensor.matmul(pden[:, 0:1], lhsT=mask[:], rhs=e_lo[:],
                                 start=False, stop=True)
                nc.tensor.matmul(pden[0:1, 1:2], lhsT=bcol, rhs=e_hi[:],
                                 start=True, stop=False)
                nc.tensor.matmul(pden[0:1, 1:2], lhsT=bcol, rhs=e_lo[:],
                                 start=False, stop=True)
                pcar = pc_pool.tile([1, D], f32)
                nc.tensor.matmul(pcar[:], lhsT=bcol, rhs=xe_hi[:],
                                 start=True, stop=False)
                nc.tensor.matmul(pcar[:], lhsT=bcol, rhs=xe_lo[:],
                                 start=False, stop=True)
                prev[ch] = (pcar, pden)

                rec = spool.tile([128, 1], f32, tag="rec")
                nc.vector.reciprocal(rec[:], pden[:, 0:1])
                outt = opool.tile([128, D], f32)
                nc.scalar.activation(outt[:], pnum[:], AF.Copy, scale=rec[:])
                nc.sync.dma_start(
                    y_d[ch].ap()[TPT * k: TPT * k + TPT, :], outt[1:128, :])

    nc.compile()
    return nc


def _shard(context, lengths, theta):
    """Host-side: cut 16 sub-slabs at segment boundaries, build padded inputs
    plus per-tile masks/bcols. Returns in_maps (one dict per core) and the
    (start, n) of each sub-slab for reassembly."""
    import ml_dtypes

    cum = np.cumsum(lengths)                      # [B] segment end (exclusive)
    assert cum[-1] == T
    # boundaries at segment edges nearest j*T/NSUB
    bounds = [0]
    for j in range(1, NSUB):
        tgt = j * (T // NSUB)
        i = np.searchsorted(cum, tgt)
        lo = cum[i - 1] if i > 0 else 0
        hi = cum[i]
        bounds.append(int(lo if tgt - lo <= hi - tgt else hi))
    bounds.append(T)

    seg_end = np.repeat(cum - 1, lengths)          # [T] global last tok of own seg

    thetab = np.tile(theta.reshape(1, D), (128, 1)).astype(np.float32)

    jj = np.arange(128)
    ii = np.arange(128)
    tri = ii[None, :] >= jj[:, None]               # [j, i] upper-tri incl diag

    in_maps = []
    slabs = []
    for c in range(NCORES):
        im = {"thetab": thetab}
        for ch in range(2):
            u = 2 * c + ch
            b0, b1 = bounds[u], bounds[u + 1]
            n = b1 - b0
            assert n <= NPAD, (u, n)
            slabs.append((b0, n))

            x_ext = np.zeros((1 + NPAD, D), dtype=np.float32)
            x_ext[1:1 + n] = context[b0:b1]

            # local end row per (tile k, row j): token g = 127k + j - 1 (local)
            loc_end = np.empty(NPAD + 1, dtype=np.int64)   # index shift: [-1..NPAD)
            loc_end[0] = -1
            loc_end[1:1 + n] = seg_end[b0:b1] - b0
            pad = np.arange(n, NPAD)
            loc_end[1 + n:] = pad                          # padding: own segment
            k_arr = np.arange(SUBTILES)
            # end_all[j, k] = min(loc_end[127k + j - 1] + 1 - 127k, 127)
            idx = TPT * k_arr[None, :] + jj[:, None]       # = (g local) + 1
            end_all = np.minimum(loc_end[idx] + 1 - TPT * k_arr[None, :], 127)

            masks = np.empty((SUBTILES, 128, 128), dtype=ml_dtypes.bfloat16)
            for k in range(SUBTILES):
                m = tri & (ii[None, :] <= end_all[:, k][:, None])
                masks[k] = m
            bcols = (end_all == 127).astype(ml_dtypes.bfloat16)  # [128, SUBTILES]

            im[f"x{ch}"] = x_ext
            im[f"masks{ch}"] = masks
            im[f"bcols{ch}"] = bcols
        in_maps.append(im)
    return in_maps, slabs


def kernel(context, context_theta, lengths, seg_ids):
    from concourse.bass_utils import run_bass_kernel_spmd

    context = np.asarray(context, dtype=np.float32)
    theta = np.asarray(context_theta, dtype=np.float32)
    lengths = np.asarray(lengths).astype(np.int64)

    if "nc" not in _CACHE:
        _CACHE["nc"] = _build_program()
    nc = _CACHE["nc"]

    in_maps, slabs = _shard(context, lengths, theta)
    res = run_bass_kernel_spmd(nc, in_maps, list(range(NCORES)))
    _CACHE["last_results"] = res

    out = np.empty((T, D), dtype=np.float32)
    for c in range(NCORES):
        for ch in range(2):
            b0, n = slabs[2 * c + ch]
            out[b0:b0 + n] = res.results[c][f"y{ch}"][:n]
    return out


# revision 3
# speedup vs baseline: 2.1389x; 2.1389x over previous
"""Trainium2 Bass kernel for ragged-sequence growing-prefix softmax attention.

Reference computation (T=131072 tokens, B=1024 ragged segments, D=512):
    s = context @ theta            # [T] scores; |s| <= ~0.07 for this data
    e = exp(s - segmax)            # segmax cancels exactly in the ratio
    out_t = segprefix(e*c)_t / segprefix(e)_t

Device strategy (8 cores, data parallel over segments):
  - 16 sub-slabs cut at segment boundaries near j*8192 tokens; core c gets
    sub-slabs 2c, 2c+1 as two independent carry chains (hides serial carry
    latency under DMA).
  - Each chain: 66 tiles of 127 tokens + carry row (row 0), grouped 6 tiles
    per DMA (12KB descriptors; small per-partition descriptors otherwise cap
    the DMA queues at ~50 GB/s).
  - Host sends x as a packed bf16 hi/lo pair (same bytes as fp32) with a
    per-tile "ones" column; exp weights are folded into the MASK instead of
    the rhs:  maske[j,i] = (i>=j)&(i<=end_j) * e_j  (one tensor_scalar op),
    split into bf16 hi/lo for full-rate PE matmuls at ~fp32 precision:
      num = mhi.T@x_hi + mhi.T@x_lo + mlo.T@x_hi       (mlo.x_lo ~2^-18, dropped)
      den = mhi.T@ones_hi + mhi.T@ones_lo + mlo.T@ones_hi
  - mask column 0 = (end_j==127)*e_j extracts the running segment sum into
    psum row 0 (no extra matmul); it is re-injected as row 0 of the next
    tile's x_hi/x_lo (+ den in the ones column) via one ACT + one DVE op.
  - scores: s = reduce(x_hi * theta) per 6-tile group (bf16, adequate: s
    error ~1e-4 -> output error well under the fp32 reference's own
    cancellation noise (max 5.2e-3 / p99 5.3e-4 vs float64)).
"""
import numpy as np

T = 131072
B = 1024
D = 512
NCORES = 8
NSUB = 16               # sub-slabs (2 per core)
TPT = 127               # tokens per tile (row 0 is the carry row)
SUBTILES = 66           # tiles per sub-slab
GT = 6                  # tiles per DMA group
NG = SUBTILES // GT     # 11 groups
CW = 520                # per-tile block width: 512 x | 1 ones | 7 pad
W = GT * CW             # 3120  per-group packed width (per hi or lo half)
NPAD = TPT * SUBTILES   # 8382 padded tokens per sub-slab

_CACHE = {}


def _build_program():
    import concourse.bacc as bacc
    import concourse.tile as tile
    import concourse.mybir as mybir
    from contextlib import ExitStack

    f32 = mybir.dt.float32
    bf16 = mybir.dt.bfloat16
    AF = mybir.ActivationFunctionType
    ALU = mybir.AluOpType

    nc = bacc.Bacc("TRN2", target_bir_lowering=False, debug=False)

    x_d = [nc.dram_tensor(f"x{ch}", [NG, 128, 2 * W], bf16, kind="ExternalInput")
           for ch in range(2)]
    e_d = [nc.dram_tensor(f"end{ch}", [128, SUBTILES], f32, kind="ExternalInput")
           for ch in range(2)]
    iota_d = nc.dram_tensor("iota_mod", [128, 128], f32, kind="ExternalInput")
    th_d = nc.dram_tensor("thetab", [128, W], bf16, kind="ExternalInput")
    y_d = [nc.dram_tensor(f"y{ch}", [NG, 128, GT * D], f32, kind="ExternalOutput")
           for ch in range(2)]

    with tile.TileContext(nc) as tc, ExitStack() as ctx:
        cpool = ctx.enter_context(tc.tile_pool(name="consts", bufs=1))
        xpool = ctx.enter_context(tc.tile_pool(name="x", bufs=3))
        spool = ctx.enter_context(tc.tile_pool(name="scr", bufs=2))
        gpool = ctx.enter_context(tc.tile_pool(name="gsmall", bufs=3))
        mpool = ctx.enter_context(tc.tile_pool(name="mask", bufs=3))
        opool = ctx.enter_context(tc.tile_pool(name="out", bufs=3))
        pspool = ctx.enter_context(tc.tile_pool(name="ps", bufs=3, space="PSUM"))

        iota = cpool.tile([128, 128], f32)
        nc.sync.dma_start(iota[:], iota_d.ap()[:])
        thetab = cpool.tile([128, W], bf16)
        nc.sync.dma_start(thetab[:], th_d.ap()[:])
        end_sb = [cpool.tile([128, SUBTILES], f32, name=f"end_sb{ch}",
                             tag=f"end{ch}") for ch in range(2)]
        for ch in range(2):
            nc.sync.dma_start(end_sb[ch][:], e_d[ch].ap()[:])

        prev = [None, None]   # previous tile's psum (carry source) per chain

        for g in range(NG):
            for ch in range(2):
                xt = xpool.tile([128, 2 * W], bf16)
                nc.sync.dma_start(xt[:], x_d[ch].ap()[g])

                # scores for the whole group: s = sum(x_hi * theta) per block
                scr = spool.tile([128, W], bf16)
                nc.vector.tensor_tensor(scr[:], xt[:, 0:W], thetab[:],
                                        op=ALU.mult)
                s_g = gpool.tile([128, GT], f32, tag="sg")
                nc.vector.tensor_reduce(
                    s_g[:], scr[:].rearrange("p (t c) -> p t c", c=CW),
                    axis=mybir.AxisListType.X, op=ALU.add)
                e32 = gpool.tile([128, GT], f32, tag="e32")
                nc.scalar.activation(e32[:], s_g[:], AF.Exp)
                # carry pseudo-row weight is 1.0 (carry is already e-weighted)
                nc.vector.memset(e32[0:1, :], 1.0)

                y_g = opool.tile([128, GT * D], f32)

                for t in range(GT):
                    k = GT * g + t
                    xhi = xt[:, t * CW: t * CW + D]
                    ones_hi = xt[:, t * CW + D: t * CW + D + 1]
                    xlo = xt[:, W + t * CW: W + t * CW + D]
                    ones_lo = xt[:, W + t * CW + D: W + t * CW + D + 1]
                    ecol = e32[:, t: t + 1]
                    endc = end_sb[ch][:, k: k + 1]

                    # carry inject from previous tile (same chain)
                    if prev[ch] is not None:
                        ppsum = prev[ch]
                        nc.scalar.copy(xt[0:1, t * CW: t * CW + D + 1],
                                       ppsum[0:1, 0: D + 1])
                        nc.vector.tensor_tensor(
                            xt[0:1, W + t * CW: W + t * CW + D + 1],
                            ppsum[0:1, 0: D + 1],
                            xt[0:1, t * CW: t * CW + D + 1],
                            op=ALU.subtract)

                    # maske[j,i] = (i>=j & i<=end_j) * e_j ; col0 = (end==127)*e_j
                    maske = mpool.tile([128, 128], f32, tag="maske")
                    nc.vector.tensor_scalar(maske[:], iota[:], endc, ecol,
                                            op0=ALU.is_le, op1=ALU.mult)
                    nc.gpsimd.tensor_scalar(maske[:, 0:1], endc, 127.0, ecol,
                                            op0=ALU.is_equal, op1=ALU.mult)
                    mhi = mpool.tile([128, 128], bf16, tag="mhi")
                    nc.gpsimd.tensor_copy(mhi[:], maske[:])
                    mlo = mpool.tile([128, 128], bf16, tag="mlo")
                    nc.vector.tensor_tensor(mlo[:], maske[:], mhi[:],
                                            op=ALU.subtract)

                    # psum: [:, 0:512] = num, [:, 512:513] = den (next bank)
                    psum = pspool.tile([128, 1024], f32)
                    nc.tensor.matmul(psum[:, 0:D], lhsT=mhi[:], rhs=xhi,
                                     start=True, stop=False)
                    nc.tensor.matmul(psum[:, 0:D], lhsT=mhi[:], rhs=xlo,
                                     start=False, stop=False)
                    nc.tensor.matmul(psum[:, D:D + 1], lhsT=mhi[:], rhs=ones_hi,
                                     start=True, stop=False)
                    nc.tensor.matmul(psum[:, D:D + 1], lhsT=mhi[:], rhs=ones_lo,
                                     start=False, stop=False)
                    nc.tensor.matmul(psum[:, 0:D], lhsT=mlo[:], rhs=xhi,
                                     start=False, stop=True)
                    nc.tensor.matmul(psum[:, D:D + 1], lhsT=mlo[:], rhs=ones_hi,
                                     start=False, stop=True)
                    prev[ch] = psum

                    rec = gpool.tile([128, 1], f32, tag="rec")
                    nc.vector.reciprocal(rec[:], psum[:, D:D + 1])
                    nc.scalar.activation(y_g[:, t * D:(t + 1) * D],
                                         psum[:, 0:D], AF.Copy, scale=rec[:])

                nc.scalar.dma_start(y_d[ch].ap()[g], y_g[:])

    nc.compile()
    return nc


def _bounds(lengths):
    cum = np.cumsum(lengths)
    assert cum[-1] == T
    bounds = [0]
    for j in range(1, NSUB):
        tgt = j * (T // NSUB)
        i = np.searchsorted(cum, tgt)
        lo = cum[i - 1] if i > 0 else 0
        hi = cum[i]
        bounds.append(int(lo if tgt - lo <= hi - tgt else hi))
    bounds.append(T)
    return bounds, cum


def _shard(context, lengths, theta):
    """Build per-core input maps: packed bf16 hi/lo x groups, end tables,
    iota_mod constant, replicated theta."""
    import ml_dtypes

    bounds, cum = _bounds(lengths)
    seg_end = np.repeat(cum - 1, lengths)     # [T] global last token of own seg

    jj = np.arange(128)
    iota_mod = np.where(jj[None, :] >= jj[:, None],
                        jj[None, :].astype(np.float32), 1e9).astype(np.float32)

    thetab = np.zeros((128, W), dtype=ml_dtypes.bfloat16)
    th = theta.reshape(-1).astype(ml_dtypes.bfloat16)
    for t in range(GT):
        thetab[:, t * CW: t * CW + D] = th[None, :]

    in_maps = []
    slabs = []
    for c in range(NCORES):
        im = {"thetab": thetab, "iota_mod": iota_mod}
        for ch in range(2):
            u = 2 * c + ch
            b0, b1 = bounds[u], bounds[u + 1]
            n = b1 - b0
            assert n <= NPAD, (u, n)
            slabs.append((b0, n))

            x_ext = np.zeros((1 + NPAD, D), dtype=np.float32)
            x_ext[1:1 + n] = context[b0:b1]
            # tile k row p holds token 127k + p - 1 -> x_ext row 127k + p
            rows = (TPT * np.arange(SUBTILES))[:, None] + jj[None, :]  # [66,128]
            xg = x_ext[rows]                          # [66, 128, 512] fp32
            x_hi = xg.astype(ml_dtypes.bfloat16)
            x_lo = (xg - x_hi.astype(np.float32)).astype(ml_dtypes.bfloat16)

            xpk = np.zeros((NG, 128, 2 * W), dtype=ml_dtypes.bfloat16)
            hi = xpk[:, :, 0:W].reshape(NG, 128, GT, CW)
            lo = xpk[:, :, W:2 * W].reshape(NG, 128, GT, CW)
            # [66,128,512] -> [NG, GT, 128, 512] -> [NG, 128, GT, 512]
            hi[:, :, :, 0:D] = x_hi.reshape(NG, GT, 128, D).transpose(0, 2, 1, 3)
            lo[:, :, :, 0:D] = x_lo.reshape(NG, GT, 128, D).transpose(0, 2, 1, 3)
            hi[:, :, :, D] = 1.0

            loc_end = np.empty(NPAD + 1, dtype=np.int64)
            loc_end[0] = -1
            loc_end[1:1 + n] = seg_end[b0:b1] - b0
            loc_end[1 + n:] = np.arange(n, NPAD)
            k_arr = np.arange(SUBTILES)
            idx = TPT * k_arr[None, :] + jj[:, None]
            end_all = np.minimum(loc_end[idx] + 1 - TPT * k_arr[None, :],
                                 127).astype(np.float32)

            im[f"x{ch}"] = xpk
            im[f"end{ch}"] = end_all
        in_maps.append(im)
    return in_maps, slabs


def kernel(context, context_theta, lengths, seg_ids):
    from concourse.bass_utils import run_bass_kernel_spmd

    context = np.asarray(context, dtype=np.float32)
    theta = np.asarray(context_theta, dtype=np.float32)
    lengths = np.asarray(lengths).astype(np.int64)

    if "nc" not in _CACHE:
        _CACHE["nc"] = _build_program()
    nc = _CACHE["nc"]

    in_maps, slabs = _shard(context, lengths, theta)
    res = run_bass_kernel_spmd(nc, in_maps, list(range(NCORES)))
    _CACHE["last_results"] = res

    out = np.empty((T, D), dtype=np.float32)
    for c in range(NCORES):
        for ch in range(2):
            b0, n = slabs[2 * c + ch]
            ypk = res.results[c][f"y{ch}"]            # [NG, 128, GT*D]
            y = ypk.reshape(NG, 128, GT, D).transpose(0, 2, 1, 3)  # [NG,GT,128,D]
            y = y.reshape(SUBTILES, 128, D)[:, 1:, :].reshape(NPAD, D)
            out[b0:b0 + n] = y[:n]
    return out
